# revision 1
# baseline (speedup 1.0000x reference)
# Trainium2 Bass kernel for DenseFeatureNumericEmbedding.
#
# Math (per batch row b, feature f):
#   h[b,f,:]  = relu(x[b,f] * W1[f,:] + b1[f,:])          # Linear(1,H) + ReLU
#   emb[b,f,:] = W2[f] @ h[b,f,:] + b2[f,:]               # Linear(H,E)
#   out[b]    = concat_f emb[b,f,:]                       # [B, F*E]
#
# Shapes: B=16384, F=128, H=64, E=16.  8 NeuronCores, batch-sharded (2048 rows/core).
#
# Device pipeline per core (per 1024-row chunk, per feature-pair j = (2j, 2j+1)):
#   1. x ships pre-transposed from host as fp8 e4m3 hi/lo components (x
#      pre-scaled by 32): xt [128 feat, 2 comp, b] in SBUF, straight DMA.
#   2. L1 "broadcast" matmul in fp8 DoubleRow perf mode: K=2 selector
#      stationary [2, 2comp, 128] (rows = the pair's two features) x moving
#      xt[2j:2j+2, :, :] -> PSUM [128p = (2 feats x 64 h-slots), b] fp32
#      = 32*(x_hi + x_lo), at 2 moving columns/cycle.
#   3. Fused drain at FD=1024 (three engines, per-pair static assignment):
#        ACT:  h = relu(scale[p]*x + bias[p])             (scale = W1/32)
#        DVE/POOL: h = max((W1/32)[p]*x, -b1[p]) = relu(W1 x + b1) - b1
#                  (residual folded into b2)
#      -> h tiles [128, 1024] bf16 in SBUF.
#   4. L2 matmul: stationary block-diag W2 pair [K=128, M=32] bf16,
#      tile_position col-packed, 4 pairs x 2 halves -> PSUM [128p = 8f x 16e, b].
#   5. Bias drain (ACT Identity w/ per-partition b2 bias) -> ot [fe, b] bf16;
#      PE transpose (bf16 PSUM) 128x128 blocks; DVE 2x tensor_copy ->
#      out_sb [b, fe] bf16; DMA column-slabs to DRAM bf16; host upcasts.

import numpy as np
import ml_dtypes

BF16 = ml_dtypes.bfloat16
FP8 = ml_dtypes.float8_e4m3  # TRN float8e4: IEEE e4m3, max normal 240

B, F, H, E = 16384, 128, 64, 16
NCORES = 8
BC = B // NCORES            # rows per core
CH = 1024                   # batch columns per chunk
FE = F * E                  # output width
NPAIR = F // 2              # feature pairs
NGROUP = F // 8             # groups of 8 features (one out-psum tile each)

X_SCALE = 32.0              # keep |x|*32 < 240 (e4m3 max normal)

# h-drain engine per pair index j: 'a' = ACT relu, 'd' = DVE max-trick.
# (GPSIMD cannot access PSUM on TRN2, so Pool can't help with drains.)
# a=31, d=33 of 64: ACT also takes the 32 bias drains, DVE the 32 2x copies.
_PAT64 = list("dadadadadadadadadadadadadadadadaddadadadadadadadadadadadadadadad")
assert len(_PAT64) == 64


def _drain_engine(j):
    return {"a": "act", "d": "dve"}[_PAT64[j % 64]]


def _pack_weights(W1, b1, W2, b2):
    W1 = np.asarray(W1, np.float32)
    b1 = np.asarray(b1, np.float32)
    W2 = np.asarray(W2, np.float32)
    b2 = np.asarray(b2, np.float32)

    # Per-partition L1 scale/bias columns: partition p of pair j holds
    # (feature 2j + p//64, h = p%64). PSUM holds 32*x so scale = W1/32.
    scl = np.zeros((128, NPAIR), np.float32)
    bia = np.zeros((128, NPAIR), np.float32)
    for j in range(NPAIR):
        scl[:64, j] = W1[2 * j] / X_SCALE
        scl[64:, j] = W1[2 * j + 1] / X_SCALE
        bia[:64, j] = b1[2 * j]
        bia[64:, j] = b1[2 * j + 1]

    # L2 stationaries: block-diag per pair, [K=128 (2x64 h), M=32 (2x16 e)].
    w2sb = np.zeros((128, NPAIR * 32), np.float32)
    for j in range(NPAIR):
        w2sb[:64, 32 * j : 32 * j + 16] = W2[2 * j].T          # [H, E]
        w2sb[64:, 32 * j + 16 : 32 * j + 32] = W2[2 * j + 1].T

    # DVE/POOL-drained pairs produce h' = relu(.) - b1; fold the residual
    # sum_h W2[f,e,h]*b1[f,h] back into the output bias.
    resid = np.einsum("feh,fh->fe", W2, b1)
    b2adj = b2.copy()
    for f in range(F):
        if _drain_engine(f // 2) != "act":
            b2adj[f] += resid[f]

    # Output bias columns: partition p of group g = (q=p//32, d=(p%32)//16, e=p%16)
    # -> feature 8g + 2q + d.
    b2col = np.zeros((128, NGROUP), np.float32)
    for g in range(NGROUP):
        for q in range(4):
            for d in range(2):
                f = 8 * g + 2 * q + d
                lo = 32 * q + 16 * d
                b2col[lo : lo + 16, g] = b2adj[f]

    # L1 DoubleRow selectors, one [K=128, 2comp, M=128] 0/1 matrix per pair:
    # row 2j -> out cols 0..63 (feature 2j), row 2j+1 -> cols 64..127, both
    # fp8 components weighted 1.0.
    sel2 = np.zeros((128, NPAIR, 2, 128), np.float32)
    for j in range(NPAIR):
        sel2[2 * j, j, :, :64] = 1.0
        sel2[2 * j + 1, j, :, 64:] = 1.0

    ident = np.eye(128, dtype=np.float32)
    return dict(
        scl=scl,
        bia=bia,
        bianeg=-bia,
        w2sb=w2sb.astype(BF16),
        b2col=b2col,
        sel2=sel2.astype(FP8),
        identf=ident.astype(BF16),
    )


def _prep_x(xs):
    """Per-core x [BC, F] fp32 -> [128 feat, 2 comp, BC] fp8 e4m3 of 32*x,
    comp-major (hi plane then lo plane per feature row)."""
    xt = np.asarray(xs, np.float32).T * X_SCALE        # [F, BC]
    hi = xt.astype(FP8)
    lo = (xt - hi.astype(np.float32)).astype(FP8)
    xp = np.empty((F, 2, xt.shape[1]), FP8)
    xp[:, 0, :] = hi
    xp[:, 1, :] = lo
    return xp


def _build(nrows):
    from contextlib import ExitStack
    import concourse.bacc as bacc
    import concourse.mybir as mybir
    import concourse.tile as tile

    dt = mybir.dt
    AF = mybir.ActivationFunctionType
    ALU = mybir.AluOpType
    DR = mybir.MatmulPerfMode.DoubleRow

    nchunk = nrows // CH
    nsub = CH // 128            # 128-row sub-blocks per chunk
    nc = bacc.Bacc(None, target_bir_lowering=False)

    xp_d = nc.declare_dram_parameter("xp", [F, 2, nrows], dt.float8e4, isOutput=False)
    scl_d = nc.declare_dram_parameter("scl", [128, NPAIR], dt.float32, isOutput=False)
    bia_d = nc.declare_dram_parameter("bia", [128, NPAIR], dt.float32, isOutput=False)
    bianeg_d = nc.declare_dram_parameter("bianeg", [128, NPAIR], dt.float32, isOutput=False)
    w2sb_d = nc.declare_dram_parameter("w2sb", [128, NPAIR * 32], dt.bfloat16, isOutput=False)
    b2col_d = nc.declare_dram_parameter("b2col", [128, NGROUP], dt.float32, isOutput=False)
    sel2_d = nc.declare_dram_parameter("sel2", [128, NPAIR, 2, 128], dt.float8e4, isOutput=False)
    identf_d = nc.declare_dram_parameter("identf", [128, 128], dt.bfloat16, isOutput=False)
    out_d = nc.declare_dram_parameter("out", [nrows, FE], dt.bfloat16, isOutput=True)

    with tile.TileContext(nc) as tc, ExitStack() as ctx:
        const = ctx.enter_context(tc.tile_pool(name="const", bufs=1))
        xt_p = ctx.enter_context(tc.tile_pool(name="xt", bufs=2))
        h_p = ctx.enter_context(tc.tile_pool(name="h", bufs=10))
        ot_p = ctx.enter_context(tc.tile_pool(name="ot", bufs=4))
        outsb_p = ctx.enter_context(tc.tile_pool(name="outsb", bufs=2))
        # PSUM budget (8 banks): ps_x 2x[128,1024]f32 = 4, ps_o 1x[128,2,512]f32
        # = 2, ps_t2 2x[128,8,128]bf16 = 2.
        ps_x = ctx.enter_context(tc.tile_pool(name="ps_x", bufs=2, space="PSUM"))
        ps_o = ctx.enter_context(tc.tile_pool(name="ps_o", bufs=1, space="PSUM"))
        ps_t2 = ctx.enter_context(tc.tile_pool(name="ps_t2", bufs=2, space="PSUM"))

        sclT = const.tile([128, NPAIR], dt.float32, tag="scl")
        biaT = const.tile([128, NPAIR], dt.float32, tag="bia")
        bianegT = const.tile([128, NPAIR], dt.float32, tag="bianeg")
        w2T = const.tile([128, NPAIR * 32], dt.bfloat16, tag="w2")
        b2colT = const.tile([128, NGROUP], dt.float32, tag="b2col")
        sel2T = const.tile([128, NPAIR, 2, 128], dt.float8e4, tag="sel2")
        identfT = const.tile([128, 128], dt.bfloat16, tag="identf")
        nc.sync.dma_start(sel2T[:], sel2_d[:])
        nc.sync.dma_start(sclT[:], scl_d[:])
        nc.sync.dma_start(biaT[:], bia_d[:])
        nc.sync.dma_start(bianegT[:], bianeg_d[:])

        # Prefetch all x chunks up front (straight DMA, host pre-transposed),
        # partition-sliced across 4 DGE rings so the first chunk lands fast.
        xts = []
        for c in range(nchunk):
            xt = xt_p.tile([128, 2, CH], dt.float8e4, tag="xt")
            nc.scalar.dma_start(xt[:], xp_d[:, :, c * CH : (c + 1) * CH])
            xts.append(xt)
            if c == 0:
                # consts not needed until the first out phase
                nc.sync.dma_start(w2T[:], w2sb_d[:])
                nc.sync.dma_start(b2colT[:], b2col_d[:])
                nc.sync.dma_start(identfT[:], identf_d[:])

        for c in range(nchunk):
            xt = xts[c]

            out_sb = outsb_p.tile([128, nsub, NGROUP, 128], dt.bfloat16, tag="out_sb")

            def out_l2(g, hts):
                # L2 matmuls (q-outer / half-inner: one stationary load serves
                # both 512-halves) + bias drain to [fe, b] SBUF tiles.
                po = ps_o.tile([128, 2, 512], dt.float32, tag="ps_out")
                for q in range(4):
                    j = 4 * g + q
                    for half in range(2):
                        nc.tensor.matmul(
                            po[32 * q : 32 * q + 32, half, :],
                            w2T[:, 32 * j : 32 * j + 32],
                            hts[q][:, 512 * half : 512 * (half + 1)],
                            start=True,
                            stop=True,
                            tile_position=(0, 32 * q),
                        )
                # single FD=1024 bias drain over both halves (ACT Identity
                # supports a per-partition bias AP and shares Relu's table)
                ot = ot_p.tile([128, 2, 512], dt.bfloat16, tag="ot")
                nc.scalar.activation(
                    ot[:], po[:], AF.Identity, bias=b2colT[:, g : g + 1], scale=1.0
                )
                return ot

            def out_tr(g, ot):
                # PE transpose (bf16 psum) to [b, fe] + one FD=1024 DVE 2x
                # copy into out_sb.
                pst = ps_t2.tile([128, 8, 128], dt.bfloat16, tag="ps_ot")
                for t8 in range(8):
                    nc.tensor.transpose(
                        pst[:, t8, :],
                        ot[:, t8 // 4, 128 * (t8 % 4) : 128 * (t8 % 4 + 1)],
                        identfT[:],
                    )
                dst = out_sb[:, :, g, :]
                nc.vector.tensor_copy(dst, pst[:])

            def ship(g):
                # column-slab DMAs as soon as a slab's last group is in out_sb
                bounds = {NGROUP // 2 - 1: (0, NGROUP // 2),
                          3 * NGROUP // 4 - 1: (NGROUP // 2, 3 * NGROUP // 4),
                          7 * NGROUP // 8 - 1: (3 * NGROUP // 4, 7 * NGROUP // 8),
                          NGROUP - 1: (7 * NGROUP // 8, NGROUP)}
                if g in bounds:
                    lo, hi = bounds[g]
                    for t in range(nsub):
                        r0 = c * CH + t * 128
                        nc.sync.dma_start(
                            out_d[r0 : r0 + 128, 128 * lo : 128 * hi],
                            out_sb[:, t, lo:hi, :],
                        )

            pend_l2 = None
            pend_tr = None
            for g in range(NGROUP):
                hts = []
                for q in range(4):
                    j = 4 * g + q
                    ps = ps_x.tile([128, CH], dt.float32, tag="ps_x")
                    sel = sel2T[:, j, :, :]
                    nc.tensor.matmul(
                        ps[:, 0:512],
                        sel,
                        xt[:, :, 0:512],
                        start=True,
                        stop=True,
                        perf_mode=DR,
                    )
                    nc.tensor.matmul(
                        ps[:, 512:1024],
                        sel,
                        xt[:, :, 512:1024],
                        start=True,
                        stop=True,
                        perf_mode=DR,
                    )
                    ht = h_p.tile([128, CH], dt.bfloat16, tag="h")
                    eng = _drain_engine(j)
                    if eng == "act":
                        nc.scalar.activation(
                            ht[:],
                            ps[:],
                            AF.Relu,
                            bias=biaT[:, j : j + 1],
                            scale=sclT[:, j : j + 1],
                        )
                    else:
                        nc.vector.tensor_scalar(
                            ht[:],
                            ps[:],
                            sclT[:, j : j + 1],
                            bianegT[:, j : j + 1],
                            ALU.mult,
                            ALU.max,
                        )
                    hts.append(ht)
                if pend_l2 is not None:
                    gl, hl = pend_l2
                    ots = out_l2(gl, hl)
                    if pend_tr is not None:
                        gt, ot_prev = pend_tr
                        out_tr(gt, ot_prev)
                        ship(gt)
                    pend_tr = (gl, ots)
                pend_l2 = (g, hts)
            # epilogue: flush both pipeline stages
            gl, hl = pend_l2
            ots = out_l2(gl, hl)
            if pend_tr is not None:
                gt, ot_prev = pend_tr
                out_tr(gt, ot_prev)
                ship(gt)
            out_tr(gl, ots)
            ship(gl)

    nc.compile()
    return nc


_NC_CACHE = {}


def _get_program(nrows):
    if nrows not in _NC_CACHE:
        _NC_CACHE[nrows] = _build(nrows)
    return _NC_CACHE[nrows]


def kernel(x, W1, b1, W2, b2, _trace=False):
    from concourse.bass_utils import run_bass_kernel_spmd

    x = np.asarray(x, np.float32)
    cfg = _pack_weights(W1, b1, W2, b2)
    nc = _get_program(BC)
    wkeys = ("scl", "bia", "bianeg", "w2sb", "b2col", "sel2", "identf")
    in_maps = []
    for c in range(NCORES):
        m = {"xp": _prep_x(x[c * BC : (c + 1) * BC])}
        for k in wkeys:
            m[k] = cfg[k]
        in_maps.append(m)
    res = run_bass_kernel_spmd(
        nc, in_maps, core_ids=list(range(NCORES)), trace=_trace
    )
    out = np.concatenate(
        [np.asarray(r["out"]).astype(np.float32) for r in res.results], axis=0
    )
    if _trace:
        kernel.last_result = res
    return np.ascontiguousarray(out)



# revision 3
# speedup vs baseline: 1.0641x; 1.0641x over previous
# Trainium2 Bass kernel for DenseFeatureNumericEmbedding.
#
# Math (per batch row b, feature f):
#   h[b,f,:]  = relu(x[b,f] * W1[f,:] + b1[f,:])          # Linear(1,H) + ReLU
#   emb[b,f,:] = W2[f] @ h[b,f,:] + b2[f,:]               # Linear(H,E)
#   out[b]    = concat_f emb[b,f,:]                       # [B, F*E]
#
# Shapes: B=16384, F=128, H=64, E=16.  8 NeuronCores, batch-sharded (2048 rows/core).
#
# Device pipeline per core (per 1024-row chunk, per feature-pair j = (2j, 2j+1)):
#   1. x ships pre-transposed from host as fp8 e4m3 hi/lo components (x
#      pre-scaled by 32): xt [128 feat, 2 comp, b] in SBUF, straight DMA.
#   2. L1 "broadcast" matmul in fp8 DoubleRow perf mode: K=2 selector
#      stationary [2, 2comp, 128] (rows = the pair's two features) x moving
#      xt[2j:2j+2, :, :] -> PSUM [128p = (2 feats x 64 h-slots), b] fp32
#      = 32*(x_hi + x_lo), at 2 moving columns/cycle.
#   3. Fused drain at FD=1024 (ACT + DVE, static split tuned for engine balance):
#        ACT:  h = relu(scale[p]*x + bias[p])             (scale = W1/32)
#        DVE:  h = max((W1/32)[p]*x, -b1[p]) = relu(W1 x + b1) - b1
#              (residual folded into the output bias b2adj)
#      -> h tiles [128, 1024] bf16 in SBUF.
#   4. L2 matmul: stationary block-diag W2 pair [K=128, M=32] bf16,
#      tile_position col-packed; half-outer/q-inner order so the 4 q-matmuls
#      of a group run CONCURRENTLY in distinct 32-col PE strips
#      -> PSUM [128p = 8f x 16e, 2, 512] fp32.
#   5. Evac on DVE: tensor_scalar add of per-partition b2adj column,
#      fp32 psum -> bf16 out_sb [fe, b].  NO on-device transpose: DRAM
#      output is [FE, BC] and the host transposes/upcasts when gathering.
#      (out row 128*g + p  ==  feature/embed index for psum partition p.)

import numpy as np
import ml_dtypes

BF16 = ml_dtypes.bfloat16
FP8 = ml_dtypes.float8_e4m3  # TRN float8e4: IEEE e4m3, max normal 240

B, F, H, E = 16384, 128, 64, 16
NCORES = 8
BC = B // NCORES            # rows per core
CH = 1024                   # batch columns per chunk
FE = F * E                  # output width
NPAIR = F // 2              # feature pairs
NGROUP = F // 8             # groups of 8 features (one out-psum tile each)

X_SCALE = 32.0              # keep |x|*32 < 240 (e4m3 max normal)

# h-drain engine split: DVE takes all 16 evacs/chunk (cheapest there) plus
# N_DVE h-drains; ACT takes the remaining h-drains. Tuned for balance:
#   DVE: 16*0.73 + a*1.34 us/chunk, ACT: (64-a)*1.16 us/chunk -> a ~= 25.
N_DVE = 25


def _drain_engine(j):
    # Bresenham spread of N_DVE DVE-drained pairs over the 64 pairs.
    return "dve" if ((j + 1) * N_DVE) // NPAIR > (j * N_DVE) // NPAIR else "act"


def _pack_weights(W1, b1, W2, b2):
    W1 = np.asarray(W1, np.float32)
    b1 = np.asarray(b1, np.float32)
    W2 = np.asarray(W2, np.float32)
    b2 = np.asarray(b2, np.float32)

    # Per-partition L1 scale/bias columns: partition p of pair j holds
    # (feature 2j + p//64, h = p%64). PSUM holds 32*x so scale = W1/32.
    scl = np.zeros((128, NPAIR), np.float32)
    bia = np.zeros((128, NPAIR), np.float32)
    for j in range(NPAIR):
        scl[:64, j] = W1[2 * j] / X_SCALE
        scl[64:, j] = W1[2 * j + 1] / X_SCALE
        bia[:64, j] = b1[2 * j]
        bia[64:, j] = b1[2 * j + 1]

    # L2 stationaries: block-diag per pair, [K=128 (2x64 h), M=32 (2x16 e)].
    w2sb = np.zeros((128, NPAIR * 32), np.float32)
    for j in range(NPAIR):
        w2sb[:64, 32 * j : 32 * j + 16] = W2[2 * j].T          # [H, E]
        w2sb[64:, 32 * j + 16 : 32 * j + 32] = W2[2 * j + 1].T

    # DVE-drained pairs produce h' = relu(.) - b1; fold the residual
    # sum_h W2[f,e,h]*b1[f,h] back into the output bias.
    resid = np.einsum("feh,fh->fe", W2, b1)
    b2adj = b2.copy()
    for f in range(F):
        if _drain_engine(f // 2) != "act":
            b2adj[f] += resid[f]

    # Output bias columns: partition p of group g = (q=p//32, d=(p%32)//16, e=p%16)
    # -> feature 8g + 2q + d.
    b2col = np.zeros((128, NGROUP), np.float32)
    for g in range(NGROUP):
        for q in range(4):
            for d in range(2):
                f = 8 * g + 2 * q + d
                lo = 32 * q + 16 * d
                b2col[lo : lo + 16, g] = b2adj[f]

    # L1 DoubleRow selectors, one [K=128, 2comp, M=128] 0/1 matrix per pair:
    # row 2j -> out cols 0..63 (feature 2j), row 2j+1 -> cols 64..127, both
    # fp8 components weighted 1.0.
    sel2 = np.zeros((128, NPAIR, 2, 128), np.float32)
    for j in range(NPAIR):
        sel2[2 * j, j, :, :64] = 1.0
        sel2[2 * j + 1, j, :, 64:] = 1.0

    return dict(
        scl=scl,
        bia=bia,
        bianeg=-bia,
        w2sb=w2sb.astype(BF16),
        b2col=b2col,
        sel2=sel2.astype(FP8),
    )


def _prep_x(xs):
    """Per-core x [BC, F] fp32 -> [128 feat, 2 comp, BC] fp8 e4m3 of 32*x,
    comp-major (hi plane then lo plane per feature row)."""
    xt = np.asarray(xs, np.float32).T * X_SCALE        # [F, BC]
    hi = xt.astype(FP8)
    lo = (xt - hi.astype(np.float32)).astype(FP8)
    xp = np.empty((F, 2, xt.shape[1]), FP8)
    xp[:, 0, :] = hi
    xp[:, 1, :] = lo
    return xp


def _build(nrows):
    from contextlib import ExitStack
    import concourse.bacc as bacc
    import concourse.mybir as mybir
    import concourse.tile as tile

    dt = mybir.dt
    AF = mybir.ActivationFunctionType
    ALU = mybir.AluOpType
    DR = mybir.MatmulPerfMode.DoubleRow

    nchunk = nrows // CH
    nc = bacc.Bacc(None, target_bir_lowering=False)

    xp_d = nc.declare_dram_parameter("xp", [F, 2, nrows], dt.float8e4, isOutput=False)
    scl_d = nc.declare_dram_parameter("scl", [128, NPAIR], dt.float32, isOutput=False)
    bia_d = nc.declare_dram_parameter("bia", [128, NPAIR], dt.float32, isOutput=False)
    bianeg_d = nc.declare_dram_parameter("bianeg", [128, NPAIR], dt.float32, isOutput=False)
    w2sb_d = nc.declare_dram_parameter("w2sb", [128, NPAIR * 32], dt.bfloat16, isOutput=False)
    b2col_d = nc.declare_dram_parameter("b2col", [128, NGROUP], dt.float32, isOutput=False)
    sel2_d = nc.declare_dram_parameter("sel2", [128, NPAIR, 2, 128], dt.float8e4, isOutput=False)
    # Output stays [FE, BC] (feature-major); host transposes after gather.
    out_d = nc.declare_dram_parameter("out", [FE, nrows], dt.bfloat16, isOutput=True)

    with tile.TileContext(nc) as tc, ExitStack() as ctx:
        const = ctx.enter_context(tc.tile_pool(name="const", bufs=1))
        xt_p = ctx.enter_context(tc.tile_pool(name="xt", bufs=2))
        h_p = ctx.enter_context(tc.tile_pool(name="h", bufs=10))
        outsb_p = ctx.enter_context(tc.tile_pool(name="outsb", bufs=2))
        # PSUM budget (8 banks): ps_x 2x[128,1024]f32 = 4, ps_o 2x[128,2,512]f32 = 4.
        ps_x = ctx.enter_context(tc.tile_pool(name="ps_x", bufs=2, space="PSUM"))
        ps_o = ctx.enter_context(tc.tile_pool(name="ps_o", bufs=2, space="PSUM"))

        sclT = const.tile([128, NPAIR], dt.float32, tag="scl")
        biaT = const.tile([128, NPAIR], dt.float32, tag="bia")
        bianegT = const.tile([128, NPAIR], dt.float32, tag="bianeg")
        w2T = const.tile([128, NPAIR * 32], dt.bfloat16, tag="w2")
        b2colT = const.tile([128, NGROUP], dt.float32, tag="b2col")
        sel2T = const.tile([128, NPAIR, 2, 128], dt.float8e4, tag="sel2")
        nc.sync.dma_start(sel2T[:], sel2_d[:])
        nc.sync.dma_start(sclT[:], scl_d[:])
        nc.sync.dma_start(biaT[:], bia_d[:])
        nc.sync.dma_start(bianegT[:], bianeg_d[:])

        # Prefetch all x chunks up front (straight DMA, host pre-transposed).
        xts = []
        for c in range(nchunk):
            xt = xt_p.tile([128, 2, CH], dt.float8e4, tag="xt")
            nc.scalar.dma_start(xt[:], xp_d[:, :, c * CH : (c + 1) * CH])
            xts.append(xt)
            if c == 0:
                nc.sync.dma_start(w2T[:], w2sb_d[:])
                nc.sync.dma_start(b2colT[:], b2col_d[:])

        for c in range(nchunk):
            xt = xts[c]

            out_sb = outsb_p.tile([128, NGROUP, CH], dt.bfloat16, tag="out_sb")

            def l1(g):
                # L1 DR matmuls + h drains for the 4 pairs of group g.
                hts = []
                for q in range(4):
                    j = 4 * g + q
                    ps = ps_x.tile([128, CH], dt.float32, tag="ps_x")
                    sel = sel2T[:, j, :, :]
                    nc.tensor.matmul(
                        ps[:, 0:512],
                        sel,
                        xt[:, :, 0:512],
                        start=True,
                        stop=True,
                        perf_mode=DR,
                    )
                    nc.tensor.matmul(
                        ps[:, 512:1024],
                        sel,
                        xt[:, :, 512:1024],
                        start=True,
                        stop=True,
                        perf_mode=DR,
                    )
                    ht = h_p.tile([128, CH], dt.bfloat16, tag="h")
                    if _drain_engine(j) == "act":
                        nc.scalar.activation(
                            ht[:],
                            ps[:],
                            AF.Relu,
                            bias=biaT[:, j : j + 1],
                            scale=sclT[:, j : j + 1],
                        )
                    else:
                        nc.vector.tensor_scalar(
                            ht[:],
                            ps[:],
                            sclT[:, j : j + 1],
                            bianegT[:, j : j + 1],
                            ALU.mult,
                            ALU.max,
                        )
                    hts.append(ht)
                return hts

            def l2(g, hts):
                # L2: half-outer / q-inner so the 4 col-tiled matmuls run
                # concurrently in distinct 32-column PE strips.
                po = ps_o.tile([128, 2, 512], dt.float32, tag="ps_out")
                for half in range(2):
                    for q in range(4):
                        j = 4 * g + q
                        nc.tensor.matmul(
                            po[32 * q : 32 * q + 32, half, :],
                            w2T[:, 32 * j : 32 * j + 32],
                            hts[q][:, 512 * half : 512 * (half + 1)],
                            start=True,
                            stop=True,
                            tile_position=(0, 32 * q),
                        )
                # Evac on DVE: add per-partition b2 column, fp32 -> bf16 SBUF.
                nc.vector.tensor_scalar_add(
                    out_sb[:, g, :],
                    po[:],
                    b2colT[:, g : g + 1],
                )

            def ship(glo, ghi):
                # out_d[128g : 128g+128, chunk cols] <- out_sb[:, g, :]
                nc.sync.dma_start(
                    out_d[128 * glo : 128 * ghi, c * CH : (c + 1) * CH].rearrange(
                        "(g p) n -> p g n", p=128
                    ),
                    out_sb[:, glo:ghi, :],
                )

            pend = None
            for g in range(NGROUP):
                hts = l1(g)
                if pend is not None:
                    gl, hl = pend
                    l2(gl, hl)
                    if gl % 4 == 3:
                        ship(gl - 3, gl + 1)
                pend = (g, hts)
            gl, hl = pend
            l2(gl, hl)
            ship(gl - 3, gl + 1)

    nc.compile()
    return nc


_NC_CACHE = {}


def _get_program(nrows):
    if nrows not in _NC_CACHE:
        _NC_CACHE[nrows] = _build(nrows)
    return _NC_CACHE[nrows]


def kernel(x, W1, b1, W2, b2, _trace=False):
    from concourse.bass_utils import run_bass_kernel_spmd

    x = np.asarray(x, np.float32)
    cfg = _pack_weights(W1, b1, W2, b2)
    nc = _get_program(BC)
    wkeys = ("scl", "bia", "bianeg", "w2sb", "b2col", "sel2")
    in_maps = []
    for c in range(NCORES):
        m = {"xp": _prep_x(x[c * BC : (c + 1) * BC])}
        for k in wkeys:
            m[k] = cfg[k]
        in_maps.append(m)
    res = run_bass_kernel_spmd(
        nc, in_maps, core_ids=list(range(NCORES)), trace=_trace
    )
    # Device output is [FE, BC] per core; transpose/upcast on host.
    out = np.concatenate(
        [np.asarray(r["out"]).astype(np.float32).T for r in res.results], axis=0
    )
    if _trace:
        kernel.last_result = res
    return np.ascontiguousarray(out)
